# revision 10
# baseline (speedup 1.0000x reference)
"""KAN-FFN (nn_KANFFN_36472862277821) Trainium2 Bass kernel.

Math: each KAN layer  out = silu(x) @ scale_base + einsum('nig,iog->no', B(x), coef*scale_sp)
with cubic B-splines (grid_size=3, k=3) on a uniform grid over [-1, 1].

Reformulation: on the uniform extended grid with knots t_q = -3 + q*h (h=2/3),
every basis B_g(x) = M(s - g) with s = 1.5*x + 4.5 and M the cardinal cubic
B-spline:  M(t) = (1/6) * sum_r (-1)^r C(4,r) relu(t - r)^3.
Hence  sum_g B_g(x) * C[i,o,g] = sum_{q=0..9} relu(s - q)^3 * D[i,o,q]
where D folds the binomial weights into the coefficients (host-side).
Each layer becomes ONE dense matmul over an 11-channel expanded feature dim
(channel 0 = silu(x), channels 1..10 = relu(s-q)^3), fp32 end to end.

Sharding: data-parallel over tokens, 16384 tokens -> 8 cores x 2048.
"""

import sys

sys.path.insert(0, "/opt/trn_rl_repo")

import numpy as np

import concourse.bacc as bacc
import concourse.mybir as mybir
import concourse.tile as tile
from concourse import dve_ops
from concourse.bass_utils import run_bass_kernel_spmd
from concourse.dve_ops import DveOp, get_dve_sub_opcode
from concourse.dve_spec import Spec, Src0, Src1, C0, C1, C2, Zero, lower, minn, relu, sq
from concourse.dve_table_gen import dve_ver_for
from concourse.dve_uop import DveOpSpec

F32 = mybir.dt.float32
F32R = mybir.dt.float32r
AF = mybir.ActivationFunctionType

N_CORES = 8
D_MODEL = 1024
KAN_HIDDEN = 128
NTOK = 4 * 4096
NTOK_CORE = NTOK // N_CORES          # 2048
MACRO = 1024                         # tokens per macro-tile
N_MACRO = NTOK_CORE // MACRO         # 2
NCH = 7                              # silu + 6 bounded B-spline channels
S_SCALE = 1.5                        # s = 1.5*x + 4.5
S_BIAS = 4.5


# ---------------------------------------------------------------- custom DVE ops
def _register(name, spec, rd1):
    for op in dve_ops.OPS:
        if op.name == name:
            return op
    op = DveOp(name, spec, subdim=False, uops_sha={})
    dve_ops.OPS.append(op)
    opcode = dve_ops._CUSTOM_DVE_ROW_BASE + len(dve_ops.OPS) - 1
    dve_ops._SUB_OPCODE_FOR_NAME[name] = opcode
    assert opcode < 0x20
    shas = {}
    for ver in ("v3", "v4"):
        try:
            compiled = DveOpSpec(
                name=name, opcode=opcode, uops=lower(spec, ver=ver), rd1_en=rd1
            )
            shas[ver] = compiled.sha(ver)
        except Exception:
            pass
    object.__setattr__(op, "uops_sha", shas)
    return op


_r = relu(Src0 * C0 + C1)
RELU_CUBE = _register("RELU_CUBE_KAN", Spec(body=_r * sq(_r)), False)

# v_g = min(1.5*x + s0, s1 - 1.5*x): tent argument of the folded cardinal B-spline
_a = Src0 * C2
VKAN = _register("VKAN_TENT", Spec(body=minn(_a + C0, C1 - _a)), False)

# chan = relu(v)^3 + s0*relu(v-1)^3  (s0=-4): in0 = v, in1 = v-1
_r1 = relu(Src0)
_r2 = relu(Src1)
CUBE2 = _register("CUBE2_KAN", Spec(body=_r1 * sq(_r1) + (_r2 * C0) * sq(_r2)), True)


# ---------------------------------------------------------------- host-side prep
def _fold_weights(coef, scale_base, scale_sp):
    """coef [I,O,6], scale_* [I,O] -> W [7, I, O] fp32 (ch0 silu, ch1..6 = coef*sp/6)."""
    I, O, _ = coef.shape
    C = (coef.astype(np.float64) * scale_sp.astype(np.float64)[:, :, None]) / 6.0
    W = np.zeros((NCH, I, O), dtype=np.float64)
    W[0] = scale_base
    for g in range(6):
        W[1 + g] = C[:, :, g]
    return np.ascontiguousarray(W.astype(np.float32))


# ---------------------------------------------------------------- kernel build
def _build_module():
    nc = bacc.Bacc(
        "TRN2",
        target_bir_lowering=False,
        debug=False,
        enable_asserts=False,
        num_devices=N_CORES,
    )

    x_d = nc.dram_tensor("x", [D_MODEL, NTOK_CORE], F32, kind="ExternalInput")
    # w1 pre-chunked on host: [88, 128, 128], chunk = ch*8 + c -> lhsT [K=feat128, M=hid128]
    w1_d = nc.dram_tensor("w1", [NCH * 8, 128, 128], F32R, kind="ExternalInput")
    # w2: [11, 128, 1024] -> rhs tiles [K=hid128, N=1024]
    w2_d = nc.dram_tensor("w2", [NCH, 128, D_MODEL], F32R, kind="ExternalInput")
    out_d = nc.dram_tensor("out", [NTOK_CORE, D_MODEL], F32, kind="ExternalOutput")

    with tile.TileContext(nc) as tc:
        with (
            tc.tile_pool(name="wpool", bufs=1) as wpool,
            tc.tile_pool(name="work", bufs=3) as pool,
            tc.tile_pool(name="psum", bufs=2, space="PSUM") as pp,
        ):
            # resident weights
            w1_sb = wpool.tile([128, NCH * 8 * 128], F32R)
            nc.sync.dma_start(
                out=w1_sb[:].rearrange("p (n f) -> p n f", n=NCH * 8),
                in_=w1_d[:].rearrange("n p f -> p n f"),
            )
            w2_sb = wpool.tile([128, NCH * D_MODEL], F32R)
            nc.sync.dma_start(
                out=w2_sb[:].rearrange("p (n f) -> p n f", n=NCH),
                in_=w2_d[:].rearrange("n p f -> p n f"),
            )

            for mt in range(N_MACRO):
                t0 = mt * MACRO
                # ---- layer 1: x arrives pre-transposed [feat, tok]; DMA per chunk
                ps_y1 = pp.tile([128, MACRO], F32, tag="y1", bufs=2)
                n_mm1 = 8 * NCH
                mm1 = 0
                for c in range(8):
                    xT = pool.tile([128, MACRO], F32, tag="xT", bufs=4)
                    nc.sync.dma_start(
                        out=xT[:], in_=x_d[c * 128 : (c + 1) * 128, t0 : t0 + MACRO]
                    )
                    # channel 0: silu
                    sil = pool.tile([128, MACRO], F32R, tag="sil", bufs=3)
                    nc.scalar.activation(sil[:], xT[:], AF.Silu)
                    for hf in range(MACRO // 512):
                        nc.tensor.matmul(
                            ps_y1[:, hf * 512 : (hf + 1) * 512],
                            lhsT=w1_sb[:, (0 * 8 + c) * 128 : (0 * 8 + c + 1) * 128],
                            rhs=sil[:, hf * 512 : (hf + 1) * 512],
                            start=(mm1 == 0),
                            stop=(mm1 == n_mm1 - 1),
                        )
                    mm1 += 1
                    for g in range(6):
                        vg = pool.tile([128, MACRO], F32, tag="vg", bufs=3)
                        nc.vector._custom_dve(
                            VKAN, out=vg[:], in0=xT[:], s0=S_BIAS - g, s1=g - 0.5, imm2=S_SCALE
                        )
                        vm1 = pool.tile([128, MACRO], F32, tag="vm1", bufs=3)
                        nc.scalar.activation(vm1[:], vg[:], AF.Copy, bias=-1.0)
                        rq = pool.tile([128, MACRO], F32R, tag="rq", bufs=4)
                        nc.vector._custom_dve(
                            CUBE2, out=rq[:], in0=vg[:], in1=vm1[:], s0=-4.0
                        )
                        for hf in range(MACRO // 512):
                            nc.tensor.matmul(
                                ps_y1[:, hf * 512 : (hf + 1) * 512],
                                lhsT=w1_sb[:, ((1 + g) * 8 + c) * 128 : ((1 + g) * 8 + c + 1) * 128],
                                rhs=rq[:, hf * 512 : (hf + 1) * 512],
                                start=(mm1 == 0),
                                stop=(mm1 == n_mm1 - 1),
                            )
                        mm1 += 1

                # ---- layer 2 channels from y1 [128 hid, MACRO tok]
                a2 = []
                sil2 = pool.tile([128, MACRO], F32R, tag="a2", bufs=NCH + 2)
                nc.scalar.activation(sil2[:], ps_y1[:], AF.Silu)
                a2.append(sil2)
                y1_sb = pool.tile([128, MACRO], F32, tag="y1sb", bufs=2)
                nc.scalar.copy(y1_sb[:], ps_y1[:])
                for g in range(6):
                    vg = pool.tile([128, MACRO], F32, tag="vg2", bufs=3)
                    nc.vector._custom_dve(
                        VKAN, out=vg[:], in0=y1_sb[:], s0=S_BIAS - g, s1=g - 0.5, imm2=S_SCALE
                    )
                    vm1 = pool.tile([128, MACRO], F32, tag="vm12", bufs=3)
                    nc.scalar.activation(vm1[:], vg[:], AF.Copy, bias=-1.0)
                    rq = pool.tile([128, MACRO], F32R, tag="a2", bufs=NCH + 2)
                    nc.vector._custom_dve(
                        CUBE2, out=rq[:], in0=vg[:], in1=vm1[:], s0=-4.0
                    )
                    a2.append(rq)

                # ---- layer 2 matmuls: per 128-token subtile
                for kt in range(MACRO // 128):
                    ps_o = pp.tile([128, D_MODEL], F32, tag="out", bufs=2)
                    for half in range(2):
                        for ch in range(NCH):
                            nc.tensor.matmul(
                                ps_o[:, half * 512 : (half + 1) * 512],
                                lhsT=a2[ch][:, kt * 128 : (kt + 1) * 128],
                                rhs=w2_sb[:, ch * D_MODEL + half * 512 : ch * D_MODEL + (half + 1) * 512],
                                start=(ch == 0),
                                stop=(ch == NCH - 1),
                            )
                    orow = pool.tile([128, D_MODEL], F32, tag="orow", bufs=3)
                    nc.scalar.copy(orow[:], ps_o[:])
                    nc.sync.dma_start(
                        out=out_d[t0 + kt * 128 : t0 + (kt + 1) * 128, :], in_=orow[:]
                    )

    nc.compile()
    return nc


_NC_CACHE = {}


def _get_nc():
    if "nc" not in _NC_CACHE:
        _NC_CACHE["nc"] = _build_module()
    return _NC_CACHE["nc"]


def run_on_cores(x, w1, w2, trace=False, **kw):
    """x [NTOK, D], folded w1 [11,1024,128], w2 [11,128,1024]. Returns (out, results)."""
    nc = _get_nc()
    w1c = np.ascontiguousarray(
        w1.reshape(NCH, 8, 128, KAN_HIDDEN).reshape(NCH * 8, 128, KAN_HIDDEN)
    )
    shards = x.reshape(N_CORES, NTOK_CORE, D_MODEL)
    in_maps = [
        {"x": np.ascontiguousarray(shards[i].T), "w1": w1c, "w2": w2}
        for i in range(N_CORES)
    ]
    res = run_bass_kernel_spmd(nc, in_maps, core_ids=list(range(N_CORES)), trace=trace, **kw)
    out = np.concatenate([res.results[i]["out"] for i in range(N_CORES)], axis=0)
    return out, res


def kernel(x, coef1, scale_base1, scale_sp1, coef2, scale_base2, scale_sp2):
    x = np.asarray(x, dtype=np.float32)
    b, s, d = x.shape
    w1 = _fold_weights(np.asarray(coef1, np.float32), np.asarray(scale_base1, np.float32),
                       np.asarray(scale_sp1, np.float32))
    w2 = _fold_weights(np.asarray(coef2, np.float32), np.asarray(scale_base2, np.float32),
                       np.asarray(scale_sp2, np.float32))
    out, _ = run_on_cores(x.reshape(-1, d), w1, w2, trace=False)
    return out.reshape(b, s, d).astype(np.float32)
